# revision 11
# baseline (speedup 1.0000x reference)
"""Channel-attention kernel for Trainium2, SPMD across 8 NeuronCores.

Problem: x:[4,512,64,64] f32; q = wq@x+bq, k = wk@x+bk (Cq=64), v = wv@x+bv;
scores = q^T k -> [B,4096,4096]; attn = softmax(scores, -1);
out = v @ attn^T; y = gamma*out + x.

Sharding: 8 shards = 4 batches x 2 query-halves. Each core gets its batch's
x pre-rotated along the pixel axis so its 2048 queries sit in columns 0:2048
(softmax/AV are permutation-invariant over keys, so rotating keys/values is
harmless). This keeps the SPMD program identical on every core.

Per-core pipeline (all matmuls bf16 on the PE):
  1. Stacked QK projection (wq;wk as one [128,512] stationary -> M=128).
  2. V projection computed transposed: vT[m,c] = x^T wvT (+bv), with an
     appended ones-column so the softmax denominator falls out of the AV
     matmul for free.
  3. Scores computed transposed (scoresT[m,n] = k^T q), K=64 row-packed
     2x on the PE array; exp on the scalar engine -> bf16.
  4. AV in the [n,c] layout: outU^T[n,c] = sum_m expT[m,n] vT[m,c]; the
     ones-column yields d[n] in the same accumulation. Normalization and
     gamma fold into a per-partition activation scale.
  5. PE transpose back to [c,n] fused with the fp32 residual add.
"""

import numpy as np

import concourse.bass as bass
import concourse.bacc as bacc
import concourse.mybir as mybir
import concourse.tile as tile
from concourse import bass_utils, masks

B, C, W, H = 4, 512, 64, 64
N = W * H          # 4096 pixels
CQ = 64            # query/key channels
NH = N // 2        # 2048 queries per core
NCORES = 8
F32 = mybir.dt.float32
BF16 = mybir.dt.bfloat16
FP8E4 = mybir.dt.float8e4
FP8E5 = mybir.dt.float8e5
DR = mybir.MatmulPerfMode.DoubleRow
VPAD = 528   # fp8 vT pair stride, %16 == 0
AF = mybir.ActivationFunctionType

N_MT = N // 128    # 32 key tiles
N_G = NH // 512    # 4 query groups per core


def _emit(tc, x, wq, wk, wv, bqk, bv, gamma, y):
    nc = tc.nc

    with (
        tc.tile_pool(name="const", bufs=1) as const,
        tc.tile_pool(name="data", bufs=1) as data,
    ):
        # ---- constants / weights prep -------------------------------
        id_bf = const.tile([128, 128], BF16, tag="idb")
        masks.make_identity(nc, id_bf[:])
        id_f32 = const.tile([128, 128], F32, tag="idf")
        masks.make_identity(nc, id_f32[:])
        ones_f32 = const.tile([1, 128], F32, tag="ones")
        nc.gpsimd.memset(ones_f32[:], 1.0)
        nbias = const.tile([128, 1], F32, tag="nbias")
        nc.gpsimd.memset(nbias[:], -4.0)

        bqk_s = const.tile([128, 1], F32, tag="bqk")
        nc.sync.dma_start(bqk_s[:], bqk)
        bv_s = const.tile([1, C], F32, tag="bvs")
        nc.sync.dma_start(bv_s[:], bv)
        g_s = const.tile([1, 1], F32, tag="gs")
        nc.sync.dma_start(g_s[:], gamma)

        bvb = const.tile([128, C], F32, tag="bvb")
        gammab = const.tile([128, 1], F32, tag="gammab")
        wqkT = [const.tile([128, 128], BF16, tag=f"wqkT{cc}", name=f"wqkT{cc}")
                for cc in range(4)]
        wvT = [const.tile([128, C], BF16, tag=f"wvT{cc}", name=f"wvT{cc}")
               for cc in range(4)]

        with (
            tc.tile_pool(name="wstg", bufs=2) as wstg,
            tc.tile_pool(name="pool_x", bufs=1) as pool_x,
            tc.tile_pool(name="psA_wt", bufs=2, space="PSUM") as psA_wt,
            tc.tile_pool(name="psA_proj", bufs=3, space="PSUM") as psA,
        ):
            # broadcast bv -> [128, C] and gamma -> [128, 1] via K=1 matmul
            pbv = psA_wt.tile([128, C], F32, tag="wt")
            nc.tensor.matmul(pbv[:], ones_f32[:], bv_s[:], start=True, stop=True)
            nc.vector.tensor_copy(bvb[:], pbv[:])
            pg = psA_wt.tile([128, 1], F32, tag="wt")
            nc.tensor.matmul(pg[:], ones_f32[:], g_s[:], start=True, stop=True)
            nc.vector.tensor_copy(gammab[:], pg[:])

            # wq;wk stacked, converted to bf16, transposed on the PE
            wqk_f = wstg.tile([128, C], F32, tag="wqkf")
            nc.sync.dma_start(wqk_f[0:CQ, :], wq)
            nc.sync.dma_start(wqk_f[CQ:128, :], wk)
            wqkb = wstg.tile([128, C], BF16, tag="wqkb")
            nc.vector.tensor_copy(wqkb[:], wqk_f[:])
            for cc in range(4):
                pt = psA_wt.tile([128, 128], BF16, tag="wt")
                nc.tensor.transpose(pt[:], wqkb[:, cc * 128:(cc + 1) * 128], id_bf[:])
                nc.vector.tensor_copy(wqkT[cc][:], pt[:])

            # wv -> bf16 -> wvT[cchunk][:, c_out]
            wvb = []
            for r in range(4):
                wf = wstg.tile([128, C], F32, tag="wvf")
                nc.sync.dma_start(wf[:], wv[r * 128:(r + 1) * 128, :])
                wb = wstg.tile([128, C], BF16, tag="wvb", bufs=4,
                               name=f"wvb{r}")
                nc.vector.tensor_copy(wb[:], wf[:])
                wvb.append(wb)
            for cc in range(4):
                pt = psA_wt.tile([128, C], BF16, tag="wt")
                for r in range(4):
                    nc.tensor.transpose(
                        pt[:, r * 128:(r + 1) * 128],
                        wvb[r][:, cc * 128:(cc + 1) * 128],
                        id_bf[:],
                    )
                nc.vector.tensor_copy(wvT[cc][:], pt[:])

            # ---- load x, convert to bf16 (pipelined half-chunks) -----
            xb = [pool_x.tile([128, N], BF16, tag=f"xb{r}", name=f"xb{r}")
                  for r in range(4)]
            for hh in range(2):
                for r in range(4):
                    xs = wstg.tile([128, NH], F32, tag="xstg", bufs=3,
                                   name=f"xs{r}_{hh}")
                    nc.sync.dma_start(
                        xs[:], x[r * 128:(r + 1) * 128, hh * NH:(hh + 1) * NH])
                    nc.vector.tensor_copy(
                        xb[r][:, hh * NH:(hh + 1) * NH], xs[:])

            # ---- stacked QK projection over all pixels ---------------
            qkb = pool_x.tile([128, N], BF16, tag="qkb")
            for g in range(N // 512):
                ps = psA.tile([128, 512], F32, tag="proj")
                for cc in range(4):
                    nc.tensor.matmul(
                        ps[:], wqkT[cc][:], xb[cc][:, g * 512:(g + 1) * 512],
                        start=(cc == 0), stop=(cc == 3),
                    )
                nc.scalar.activation(
                    qkb[:, g * 512:(g + 1) * 512], ps[:], AF.Identity,
                    bias=bqk_s[:],
                )

            # duplicate q (queries only) and k across both partition halves
            q2 = data.tile([128, NH], BF16, tag="q2")
            k2 = data.tile([128, N], BF16, tag="k2")
            nc.sync.dma_start(q2[0:CQ, :], qkb[0:CQ, 0:NH])
            nc.sync.dma_start(q2[CQ:128, :], qkb[0:CQ, 0:NH])
            nc.sync.dma_start(k2[0:CQ, :], qkb[CQ:128, :])
            nc.sync.dma_start(k2[CQ:128, :], qkb[CQ:128, :])

            # ---- V projection, transposed, fp8e4 pair tiles with ones
            vP = [data.tile([128, 2 * VPAD], FP8E4, tag=f"vP{j}", name=f"vP{j}")
                  for j in range(N_MT // 2)]
            for mt in range(N_MT):
                ps = psA.tile([128, C], F32, tag="proj")
                for cc in range(4):
                    nc.tensor.matmul(
                        ps[:], xb[cc][:, mt * 128:(mt + 1) * 128], wvT[cc][:],
                        start=(cc == 0), stop=(cc == 3),
                    )
                j, half = divmod(mt, 2)
                base = half * VPAD
                nc.vector.tensor_add(vP[j][:, base:base + C], ps[:], bvb[:])
                nc.gpsimd.memset(vP[j][:, base + C:base + C + 1], 1.0)

        # ---- attention: scoresT -> exp -> AV -> normalize -> out -----
        with (
            tc.tile_pool(name="psB_sc", bufs=4, space="PSUM") as psB_sc,
            tc.tile_pool(name="psB_av", bufs=2, space="PSUM") as psB_av,
            tc.tile_pool(name="small", bufs=4) as small,
            tc.tile_pool(name="yout", bufs=3) as yout,
            tc.tile_pool(name="xres", bufs=3) as xres,
        ):
            def alloc_expP(g):
                return [data.tile([128, 1024], FP8E5, tag=f"expP{j}",
                                  name=f"expP{j}_{g}", bufs=2)
                        for j in range(N_MT // 2)]

            def score_pair(expP_list, g, j):
                mA, mB = 2 * j, 2 * j + 1
                q_lo = q2[0:CQ, g * 512:(g + 1) * 512]
                q_hi = q2[CQ:128, g * 512:(g + 1) * 512]
                pA = psB_sc.tile([128, 512], F32, tag="sc", name=f"pA{g}_{j}")
                pB = psB_sc.tile([128, 512], F32, tag="sc", name=f"pB{g}_{j}")
                nc.tensor.matmul(
                    pA[:], k2[0:CQ, mA * 128:(mA + 1) * 128], q_lo,
                    start=True, stop=True,
                )
                nc.tensor.matmul(
                    pB[:], k2[CQ:128, mB * 128:(mB + 1) * 128], q_hi,
                    start=True, stop=True,
                )
                nc.scalar.activation(expP_list[j][:, 0:512], pA[:], AF.Exp,
                                     bias=nbias[:])
                nc.scalar.activation(expP_list[j][:, 512:1024], pB[:], AF.Exp,
                                     bias=nbias[:])

            NJ = N_MT // 2
            expP = alloc_expP(0)
            for j in range(NJ):
                score_pair(expP, 0, j)
            for g in range(N_G):
                nxt = alloc_expP(g + 1) if g + 1 < N_G else None
                yT = [data.tile([128, 512], F32, tag=f"yT{t}", name=f"yT{t}_{g}")
                      for t in range(4)]
                for t in range(4):
                    # interleave next group's scores so ACT exps stay fed
                    if nxt is not None:
                        for j in range(t * 4, t * 4 + 4):
                            score_pair(nxt, g + 1, j)
                    av = psB_av.tile([128, 1024], F32, tag="av",
                                     name=f"av{g}_{t}")
                    for j in range(NJ):
                        lh = expP[j][:].rearrange("p (i n) -> p i n", i=2)[
                            :, :, t * 128:(t + 1) * 128]
                        vr = vP[j][:].rearrange("p (i n) -> p i n", i=2)
                        nc.tensor.matmul(
                            av[:, 0:256], lh, vr[:, :, 0:256],
                            start=(j == 0), stop=(j == NJ - 1), perf_mode=DR,
                        )
                        nc.tensor.matmul(
                            av[:, 512:769], lh, vr[:, :, 256:513],
                            start=(j == 0), stop=(j == NJ - 1), perf_mode=DR,
                        )
                    rd = small.tile([128, 1], F32, tag="rd")
                    nc.vector.reciprocal(rd[:], av[:, 768:769])
                    gsc = small.tile([128, 1], F32, tag="gsc")
                    nc.vector.tensor_mul(gsc[:], rd[:], gammab[:])
                    nc.scalar.activation(
                        yT[t][:].rearrange("p (i n) -> p i n", i=2),
                        av[:].rearrange("p (i n) -> p i n", i=2)[:, :, 0:256],
                        AF.Copy, scale=gsc[:],
                    )

                for cc in range(4):
                    pt = psB_sc.tile([128, 512], F32, tag="sc",
                                     name=f"pt{g}_{cc}")
                    for t in range(4):
                        nc.tensor.transpose(
                            pt[:, t * 128:(t + 1) * 128],
                            yT[t][:, cc * 128:(cc + 1) * 128],
                            id_f32[:],
                        )
                    xr = xres.tile([128, 512], F32, tag="xr")
                    nc.sync.dma_start(
                        xr[:],
                        x[cc * 128:(cc + 1) * 128, g * 512:(g + 1) * 512],
                    )
                    yo = yout.tile([128, 512], F32, tag="yo")
                    nc.vector.tensor_add(yo[:], pt[:], xr[:])
                    nc.sync.dma_start(
                        y[cc * 128:(cc + 1) * 128, g * 512:(g + 1) * 512], yo[:]
                    )
                expP = nxt


def build_nc():
    nc = bacc.Bacc("TRN2", target_bir_lowering=False, debug=False,
                   num_devices=NCORES)
    x = nc.dram_tensor("x", [C, N], F32, kind="ExternalInput")
    wq = nc.dram_tensor("wq", [CQ, C], F32, kind="ExternalInput")
    wk = nc.dram_tensor("wk", [CQ, C], F32, kind="ExternalInput")
    wv = nc.dram_tensor("wv", [C, C], F32, kind="ExternalInput")
    bqk = nc.dram_tensor("bqk", [128, 1], F32, kind="ExternalInput")
    bv = nc.dram_tensor("bv", [1, C], F32, kind="ExternalInput")
    gamma = nc.dram_tensor("gamma", [1, 1], F32, kind="ExternalInput")
    y = nc.dram_tensor("y", [C, NH], F32, kind="ExternalOutput")
    with tile.TileContext(nc) as tc:
        _emit(tc, x.ap(), wq.ap(), wk.ap(), wv.ap(), bqk.ap(), bv.ap(),
              gamma.ap(), y.ap())
    nc.compile()
    return nc


def make_in_maps(inputs):
    xf = np.ascontiguousarray(
        np.asarray(inputs["x"], dtype=np.float32).reshape(B, C, N))
    wq = np.ascontiguousarray(np.asarray(inputs["wq"], dtype=np.float32))
    wk = np.ascontiguousarray(np.asarray(inputs["wk"], dtype=np.float32))
    wv = np.ascontiguousarray(np.asarray(inputs["wv"], dtype=np.float32))
    bqk = np.concatenate([
        np.asarray(inputs["bq"], dtype=np.float32),
        np.asarray(inputs["bk"], dtype=np.float32),
    ]).reshape(128, 1)
    bv = np.asarray(inputs["bv"], dtype=np.float32).reshape(1, C)
    gamma = np.asarray(inputs["gamma"], dtype=np.float32).reshape(1, 1)
    in_maps = []
    for i in range(NCORES):
        b, h = divmod(i, 2)
        xr = np.roll(xf[b], -h * NH, axis=1) if h else xf[b]
        in_maps.append({
            "x": np.ascontiguousarray(xr), "wq": wq, "wk": wk, "wv": wv,
            "bqk": bqk, "bv": bv, "gamma": gamma,
        })
    return in_maps


_NC = None


def _get_nc():
    global _NC
    if _NC is None:
        _NC = build_nc()
    return _NC


def kernel(**inputs):
    nc = _get_nc()
    in_maps = make_in_maps(inputs)
    res = bass_utils.run_bass_kernel_spmd(nc, in_maps, core_ids=list(range(NCORES)))
    yf = np.empty((B, C, N), dtype=np.float32)
    for i in range(NCORES):
        b, h = divmod(i, 2)
        yf[b][:, h * NH:(h + 1) * NH] = res.results[i]["y"]
    return yf.reshape(B, C, W, H)
